# revision 3
# baseline (speedup 1.0000x reference)
"""LIF spiking-neuron layer on 8 Trainium2 NeuronCores (Bass/Tile).

Reference semantics (per neuron, T=6 steps, v0=0):
    v = v*0.5 + x_t ; s = (v >= 1.0) ; v = v - s
Output: spikes [T, B, C, H, W] float32 (values are exactly 0.0 / 1.0).

Sharding: data-parallel over batch (axis 1): 64 batches / 8 cores.
Per core the neuron field (8*128*32*32 = 1,048,576 elements) is laid
out as [128 partitions, 8192 cols], processed in 4 column blocks of
2048 with a 6-step sequential recurrence per block.

Per-core compute (bit-identical to the fp32 reference):
  state kept as h = v/2 (exact power-of-2 scale).
  u_t  = h_{t-1} + x_t       <- performed by the load DMA itself
                                (SWDGE accum_op=add, fp32)
  sh_t = (u_t >= 1) * 0.5    <- one DVE tensor_scalar (dual-op), bf16
                                out; {0, 0.5} both exact in bf16
  h_t  = (u_t * 0.5) - sh_t  <- one DVE scalar_tensor_tensor, in place
Spikes are stored as bf16 {0, 0.5} (halving HBM store traffic vs f32);
the host multiplies by 2 -> exact {0,1} float32.

Engine budget per core (cost model): DVE ~99us busy, HBM traffic
36 MiB (~102us) -> modeled e2e ~111us vs ~146us for the naive
fp32-store 3-op-per-step version.
"""

import os
import sys

import numpy as np

sys.path.insert(0, "/opt/trn_rl_repo")

import concourse.bacc as bacc
import concourse.bass as bass
import concourse.mybir as mybir
from concourse import tile
from concourse.bass_utils import run_bass_kernel_spmd

T = 6
B = 64
C = 128
H = 32
W = 32
N_CORES = 8
B_PER_CORE = B // N_CORES
N_PER_CORE = B_PER_CORE * C * H * W  # 1,048,576
P = 128
FTOT = N_PER_CORE // P               # 8192
FBLK = 2048
NBLK = FTOT // FBLK                  # 4

_COMPILED = None
LAST_RESULTS = None


def _build_program():
    nc = bacc.Bacc(None, target_bir_lowering=False, debug=False)

    f32, bf16 = mybir.dt.float32, mybir.dt.bfloat16
    x_d = nc.dram_tensor("x", [T, N_PER_CORE], f32, kind="ExternalInput")
    s_d = nc.dram_tensor("s", [T, N_PER_CORE], bf16, kind="ExternalOutput")
    x_r = x_d[:].rearrange("t (p f) -> t p f", p=P)

    with tile.TileContext(nc) as tc:
        with (
            tc.tile_pool(name="u", bufs=NBLK) as u_pool,
            tc.tile_pool(name="s6", bufs=NBLK) as s_pool,
        ):
            for blk in range(NBLK):
                c0 = blk * FBLK
                u = u_pool.tile([P, FBLK], f32, tag="u")
                # u_0 = x_0 (v0 = 0); plain HWDGE load
                nc.sync.dma_start(out=u[:], in_=x_r[0][:, c0:c0 + FBLK])
                s6 = s_pool.tile([P, T * FBLK], bf16, tag="s6")
                for t in range(T):
                    sl = s6[:, t * FBLK:(t + 1) * FBLK]
                    # sh = (u >= 1.0) * 0.5 -> bf16 {0, 0.5}
                    nc.vector.tensor_scalar(
                        out=sl, in0=u[:], scalar1=1.0, scalar2=0.5,
                        op0=mybir.AluOpType.is_ge,
                        op1=mybir.AluOpType.mult,
                    )
                    if t < T - 1:
                        # h = (u * 0.5) - sh, in place
                        nc.vector.scalar_tensor_tensor(
                            out=u[:], in0=u[:], scalar=0.5, in1=sl,
                            op0=mybir.AluOpType.mult,
                            op1=mybir.AluOpType.subtract,
                        )
                        # u_{t+1} = h + x_{t+1}: accumulate during load
                        nc.gpsimd.dma_start(
                            out=u[:], in_=x_r[t + 1][:, c0:c0 + FBLK],
                            accum_op=mybir.AluOpType.add,
                        )
                    # store this timestep's spikes right away
                    dram_ap = bass.AP(
                        s_d, t * N_PER_CORE + c0, [[FTOT, P], [1, FBLK]])
                    nc.sync.dma_start(out=dram_ap, in_=sl)
    nc.finalize()
    return nc


def kernel(input_current: np.ndarray) -> np.ndarray:
    global _COMPILED, LAST_RESULTS
    x = np.asarray(input_current, dtype=np.float32)
    assert x.shape == (T, B, C, H, W), x.shape

    if _COMPILED is None:
        _COMPILED = _build_program()
    nc = _COMPILED

    in_maps = []
    for k in range(N_CORES):
        shard = np.ascontiguousarray(
            x[:, k * B_PER_CORE:(k + 1) * B_PER_CORE]
        ).reshape(T, N_PER_CORE)
        in_maps.append({"x": shard})

    trace = bool(int(os.environ.get("LIF_TRACE", "0")))
    res = run_bass_kernel_spmd(nc, in_maps, core_ids=list(range(N_CORES)),
                               trace=trace)
    LAST_RESULTS = res

    out = np.empty((T, B, C, H, W), dtype=np.float32)
    for k in range(N_CORES):
        sh = res.results[k]["s"].astype(np.float32) * 2.0
        out[:, k * B_PER_CORE:(k + 1) * B_PER_CORE] = (
            sh.reshape(T, B_PER_CORE, C, H, W)
        )
    return out


# revision 4
# speedup vs baseline: 1.1197x; 1.1197x over previous
"""LIF spiking-neuron layer on 8 Trainium2 NeuronCores (Bass/Tile).

Reference semantics (per neuron, T=6 steps, v0=0):
    v = v*0.5 + x_t ; s = (v >= 1.0) ; v = v - s
Output: spikes [T, B, C, H, W] float32 (values are exactly 0.0 / 1.0).

Sharding: data-parallel over batch (axis 1): 64 batches / 8 cores.
Per core the neuron field (8*128*32*32 = 1,048,576 elements) is laid
out as [128 partitions, 8192 cols], processed in 4 column blocks of
2048 with a 6-step sequential recurrence per block.

Per-core compute (bit-identical to the fp32 reference):
  state kept as h = v/2 (exact power-of-2 scale).
  u_t  = h_{t-1} + x_t       <- performed by the load DMA itself
                                (SWDGE accum_op=add, fp32)
  sh_t = (u_t >= 1) * 0.5    <- one DVE tensor_scalar (dual-op), bf16
                                out; {0, 0.5} both exact in bf16
  h_t  = (u_t * 0.5) - sh_t  <- one DVE scalar_tensor_tensor, in place
Spikes are stored as fp8e4 {0, 0.5} (1/4 HBM store traffic vs f32);
the host multiplies by 2 -> exact {0,1} float32.

Engine budget per core (cost model): DVE ~99us busy, HBM traffic
36 MiB (~102us) -> modeled e2e ~111us vs ~146us for the naive
fp32-store 3-op-per-step version.
"""

import os
import sys

import numpy as np

sys.path.insert(0, "/opt/trn_rl_repo")

import concourse.bacc as bacc
import concourse.bass as bass
import concourse.mybir as mybir
from concourse import tile
from concourse.bass_utils import run_bass_kernel_spmd

T = 6
B = 64
C = 128
H = 32
W = 32
N_CORES = 8
B_PER_CORE = B // N_CORES
N_PER_CORE = B_PER_CORE * C * H * W  # 1,048,576
P = 128
FTOT = N_PER_CORE // P               # 8192
FBLK = 2048
NBLK = FTOT // FBLK                  # 4

_COMPILED = None
LAST_RESULTS = None


def _build_program():
    nc = bacc.Bacc(None, target_bir_lowering=False, debug=False)

    f32, f8 = mybir.dt.float32, mybir.dt.float8e4
    x_d = nc.dram_tensor("x", [T, N_PER_CORE], f32, kind="ExternalInput")
    s_d = nc.dram_tensor("s", [T, N_PER_CORE], f8, kind="ExternalOutput")
    x_r = x_d[:].rearrange("t (p f) -> t p f", p=P)

    with tile.TileContext(nc) as tc:
        with (
            tc.tile_pool(name="u", bufs=NBLK) as u_pool,
            tc.tile_pool(name="s6", bufs=NBLK) as s_pool,
        ):
            for blk in range(NBLK):
                c0 = blk * FBLK
                u = u_pool.tile([P, FBLK], f32, tag="u")
                # u_0 = x_0 (v0 = 0); plain HWDGE load
                nc.sync.dma_start(out=u[:], in_=x_r[0][:, c0:c0 + FBLK])
                s6 = s_pool.tile([P, T * FBLK], f8, tag="s6")
                for t in range(T):
                    sl = s6[:, t * FBLK:(t + 1) * FBLK]
                    # sh = (u >= 1.0) * 0.5 -> fp8e4 {0, 0.5}, both exact
                    nc.vector.tensor_scalar(
                        out=sl, in0=u[:], scalar1=1.0, scalar2=0.5,
                        op0=mybir.AluOpType.is_ge,
                        op1=mybir.AluOpType.mult,
                    )
                    if t < T - 1:
                        # h = (u * 0.5) - sh, in place
                        nc.vector.scalar_tensor_tensor(
                            out=u[:], in0=u[:], scalar=0.5, in1=sl,
                            op0=mybir.AluOpType.mult,
                            op1=mybir.AluOpType.subtract,
                        )
                        # u_{t+1} = h + x_{t+1}: accumulate during load
                        nc.gpsimd.dma_start(
                            out=u[:], in_=x_r[t + 1][:, c0:c0 + FBLK],
                            accum_op=mybir.AluOpType.add,
                        )
                    # store this timestep's spikes right away
                    dram_ap = bass.AP(
                        s_d, t * N_PER_CORE + c0, [[FTOT, P], [1, FBLK]])
                    nc.sync.dma_start(out=dram_ap, in_=sl)
    nc.finalize()
    return nc


def kernel(input_current: np.ndarray) -> np.ndarray:
    global _COMPILED, LAST_RESULTS
    x = np.asarray(input_current, dtype=np.float32)
    assert x.shape == (T, B, C, H, W), x.shape

    if _COMPILED is None:
        _COMPILED = _build_program()
    nc = _COMPILED

    in_maps = []
    for k in range(N_CORES):
        shard = np.ascontiguousarray(
            x[:, k * B_PER_CORE:(k + 1) * B_PER_CORE]
        ).reshape(T, N_PER_CORE)
        in_maps.append({"x": shard})

    trace = bool(int(os.environ.get("LIF_TRACE", "0")))
    res = run_bass_kernel_spmd(nc, in_maps, core_ids=list(range(N_CORES)),
                               trace=trace)
    LAST_RESULTS = res

    out = np.empty((T, B, C, H, W), dtype=np.float32)
    for k in range(N_CORES):
        sh = res.results[k]["s"].astype(np.float32) * 2.0
        out[:, k * B_PER_CORE:(k + 1) * B_PER_CORE] = (
            sh.reshape(T, B_PER_CORE, C, H, W)
        )
    return out


# revision 5
# speedup vs baseline: 1.1335x; 1.0123x over previous
"""LIF spiking-neuron layer on 8 Trainium2 NeuronCores (Bass/Tile).

Reference semantics (per neuron, T=6 steps, v0=0):
    v = v*0.5 + x_t ; s = (v >= 1.0) ; v = v - s
Output: spikes [T, B, C, H, W] float32 (values are exactly 0.0 / 1.0).

Sharding: data-parallel over batch (axis 1): 64 batches / 8 cores.
Per core the neuron field (8*128*32*32 = 1,048,576 elements) is laid
out as [128 partitions, 8192 cols], processed in 4 column blocks of
2048 with a 6-step sequential recurrence per block.

Per-core compute (bit-identical to the fp32 reference):
  state kept as h = v/2 (exact power-of-2 scale).
  u_t  = h_{t-1} + x_t       <- performed by the load DMA itself
                                (SWDGE accum_op=add, fp32)
  sh_t = (u_t >= 1) * 0.5    <- one DVE tensor_scalar (dual-op), bf16
                                out; {0, 0.5} both exact in bf16
  h_t  = (u_t * 0.5) - sh_t  <- one DVE scalar_tensor_tensor, in place
Spikes are stored as fp8e4 {0, 0.5} (1/4 HBM store traffic vs f32);
the host multiplies by 2 -> exact {0,1} float32.

Engine budget per core (cost model): DVE ~99us busy, HBM traffic
36 MiB (~102us) -> modeled e2e ~111us vs ~146us for the naive
fp32-store 3-op-per-step version.
"""

import os
import sys

import numpy as np

sys.path.insert(0, "/opt/trn_rl_repo")

import concourse.bacc as bacc
import concourse.bass as bass
import concourse.mybir as mybir
from concourse import tile
from concourse.bass_utils import run_bass_kernel_spmd

T = 6
B = 64
C = 128
H = 32
W = 32
N_CORES = 8
B_PER_CORE = B // N_CORES
N_PER_CORE = B_PER_CORE * C * H * W  # 1,048,576
P = 128
FTOT = N_PER_CORE // P               # 8192
FBLK = 2048
NBLK = FTOT // FBLK                  # 4

_COMPILED = None
LAST_RESULTS = None


def _build_program():
    nc = bacc.Bacc(None, target_bir_lowering=False, debug=False)

    f32, f8 = mybir.dt.float32, mybir.dt.float8e4
    x_d = nc.dram_tensor("x", [T, N_PER_CORE], f32, kind="ExternalInput")
    s_d = nc.dram_tensor("s", [T, N_PER_CORE], f8, kind="ExternalOutput")
    x_r = x_d[:].rearrange("t (p f) -> t p f", p=P)

    with tile.TileContext(nc) as tc:
        with (
            tc.tile_pool(name="u", bufs=NBLK) as u_pool,
            tc.tile_pool(name="s6", bufs=NBLK) as s_pool,
        ):
            for blk in range(NBLK):
                c0 = blk * FBLK
                u = u_pool.tile([P, FBLK], f32, tag="u")
                # u_0 = x_0 (v0 = 0); plain HWDGE load
                nc.sync.dma_start(out=u[:], in_=x_r[0][:, c0:c0 + FBLK])
                s6 = s_pool.tile([P, T * FBLK], f8, tag="s6")
                for t in range(T):
                    sl = s6[:, t * FBLK:(t + 1) * FBLK]
                    # sh = (u >= 1.0) * 0.5 -> fp8e4 {0, 0.5}, both exact
                    nc.vector.tensor_scalar(
                        out=sl, in0=u[:], scalar1=1.0, scalar2=0.5,
                        op0=mybir.AluOpType.is_ge,
                        op1=mybir.AluOpType.mult,
                    )
                    if t < T - 1:
                        # h = (u * 0.5) - sh, in place
                        nc.vector.scalar_tensor_tensor(
                            out=u[:], in0=u[:], scalar=0.5, in1=sl,
                            op0=mybir.AluOpType.mult,
                            op1=mybir.AluOpType.subtract,
                        )
                        # u_{t+1} = h + x_{t+1}: accumulate during load
                        nc.gpsimd.dma_start(
                            out=u[:], in_=x_r[t + 1][:, c0:c0 + FBLK],
                            accum_op=mybir.AluOpType.add,
                        )
                    # store spikes as soon as a pair of timesteps is done
                    if (t + 1) % 2 == 0:
                        tlo = t - 1
                        sb = s6[:, tlo * FBLK:(t + 1) * FBLK].rearrange(
                            "p (t f) -> p t f", t=2)
                        dram_ap = bass.AP(
                            s_d, tlo * N_PER_CORE + c0,
                            [[FTOT, P], [N_PER_CORE, 2], [1, FBLK]])
                        nc.sync.dma_start(out=dram_ap, in_=sb)
    nc.finalize()
    return nc


def kernel(input_current: np.ndarray) -> np.ndarray:
    global _COMPILED, LAST_RESULTS
    x = np.asarray(input_current, dtype=np.float32)
    assert x.shape == (T, B, C, H, W), x.shape

    if _COMPILED is None:
        _COMPILED = _build_program()
    nc = _COMPILED

    in_maps = []
    for k in range(N_CORES):
        shard = np.ascontiguousarray(
            x[:, k * B_PER_CORE:(k + 1) * B_PER_CORE]
        ).reshape(T, N_PER_CORE)
        in_maps.append({"x": shard})

    trace = bool(int(os.environ.get("LIF_TRACE", "0")))
    res = run_bass_kernel_spmd(nc, in_maps, core_ids=list(range(N_CORES)),
                               trace=trace)
    LAST_RESULTS = res

    out = np.empty((T, B, C, H, W), dtype=np.float32)
    for k in range(N_CORES):
        sh = res.results[k]["s"].astype(np.float32) * 2.0
        out[:, k * B_PER_CORE:(k + 1) * B_PER_CORE] = (
            sh.reshape(T, B_PER_CORE, C, H, W)
        )
    return out
